# revision 1
# baseline (speedup 1.0000x reference)
"""Trainium2 Bass kernel: 2-layer adjacency-gated multi-head attention encoder.

Strategy: data-parallel over batch (B=8 -> one batch element per NeuronCore,
8 cores). Weights and the adjacency mask are replicated; no collectives.

Per-core dataflow (S=1024 tokens, E=512, H=8 heads, D=64):
  - host pre-transposes x -> xT [E,S] and pre-transposes/scales weights
  - qT/kT = W^T-stationary matmuls producing [f, s] layout (f on partitions)
  - v kept natural [s, f] with a ones-column per head (denominator trick)
  - scoresT[k, sq] = kT_h^T @ qT_h  (contract d), masked by adjT, exp on ACT
  - attn@v: out[d(+1), sq] = (v_h|1)^T @ expT  -> row 64 = softmax denominator
  - normalize via K=1 broadcast matmul of 1/denom + fused scalar_tensor_tensor
  - out-proj back to [s, f], fused residual + LayerNorm (stats via accum_out)
  - PE transpose rebuilds xT for layer 2
All matmuls run in float32r (full-rate fp32 on the PE for N>=256).
"""

import math
import os

import numpy as np

import concourse.bass as bass
import concourse.bacc as bacc
import concourse.mybir as mybir
import concourse.tile as tile
from concourse import library_config
from concourse.bass_utils import run_bass_kernel_spmd
from concourse.masks import make_identity

P = 128
S = 1024
E = 512
H = 8
D = 64
L = 2
NE = E // P  # 4 e-chunks
NS = S // P  # 8 s-chunks
NSH = 2      # s halves of 512 (psum free dim)
FREE = 512
LN_EPS = 1e-5

F32 = mybir.dt.float32
BF16 = mybir.dt.bfloat16
# float32r: full fp32 data, reduced-precision multiply on the PE at 1 cyc/row
# (vs 4 cyc/row for strict fp32) when the moving dim is >= 256.
MM_DT = mybir.dt.float32r if os.environ.get("KERNEL_MM_DT", "f32r") == "f32r" else F32
AF = mybir.ActivationFunctionType
OP = mybir.AluOpType


def build_nc():
    nc = bacc.Bacc(None, target_bir_lowering=False)

    xT_d = nc.declare_dram_parameter("xT", [E, S], MM_DT, isOutput=False)
    xn_d = nc.declare_dram_parameter("xn", [S, E], F32, isOutput=False)
    wts_d = nc.declare_dram_parameter("wts", [L, 4, E, E], MM_DT, isOutput=False)
    pb_d = nc.declare_dram_parameter("pb", [L, 2, P, NE], F32, isOutput=False)
    fb_d = nc.declare_dram_parameter("fb", [L, 3, P, E], F32, isOutput=False)
    adjT_d = nc.declare_dram_parameter("adjT", [S, S], F32, isOutput=False)
    out_d = nc.declare_dram_parameter("out", [S, E], F32, isOutput=True)

    with tile.TileContext(nc) as tc:
        with (
            tc.tile_pool(name="const", bufs=1) as const_p,
            tc.tile_pool(name="adj", bufs=NS) as adj_p,
            tc.tile_pool(name="xt", bufs=1) as xt_p,
            tc.tile_pool(name="xn", bufs=1) as xn_p,
            tc.tile_pool(name="w", bufs=1) as w_p,
            tc.tile_pool(name="qk", bufs=1) as qk_p,
            tc.tile_pool(name="v", bufs=NS) as v_p,
            tc.tile_pool(name="exp", bufs=9) as exp_p,
            tc.tile_pool(name="at", bufs=1) as at_p,
            tc.tile_pool(name="small", bufs=2) as small_p,
            tc.tile_pool(name="scr", bufs=2) as scr_p,
            tc.tile_pool(name="ps_s", bufs=4, space="PSUM") as ps_s,
            tc.tile_pool(name="ps_o", bufs=2, space="PSUM") as ps_o,
            tc.tile_pool(name="ps_t", bufs=2, space="PSUM") as ps_t,
        ):
            ident = const_p.tile([P, P], F32, tag="ident")
            make_identity(nc, ident)
            eps_t = const_p.tile([P, 1], F32, tag="eps")
            nc.vector.memset(eps_t[:], float(LN_EPS))
            ones_c = const_p.tile([P, H], F32, tag="ones_c")
            nc.vector.memset(ones_c[:], 1.0)
            # partition_broadcast lives in the gpsimd "attn" library
            nc.gpsimd.load_library(library_config.attn)

            # ---- initial loads ----
            xT0 = xt_p.tile([P, NE, S], MM_DT, tag="xt")
            nc.sync.dma_start(
                out=xT0[:], in_=xT_d[:].rearrange("(c p) s -> p c s", p=P)
            )
            xn_t = xn_p.tile([P, NS, E], F32, tag="xn")
            nc.sync.dma_start(
                out=xn_t[:], in_=xn_d[:].rearrange("(c p) e -> p c e", p=P)
            )
            adj_t = []
            for kc in range(NS):
                a = adj_p.tile([P, S], F32, tag="adj")
                nc.sync.dma_start(out=a[:], in_=adjT_d[kc * P : (kc + 1) * P, :])
                adj_t.append(a)

            xT_cur = xT0
            for layer in range(L):
                # ---- per-layer weight/bias loads ----
                w_t = []
                for m in range(4):
                    w = w_p.tile([P, NE, E], MM_DT, tag=f"w{m}")
                    nc.sync.dma_start(
                        out=w[:],
                        in_=wts_d[layer, m].rearrange("(c p) f -> p c f", p=P),
                    )
                    w_t.append(w)
                pb_t = small_p.tile([P, 2, NE], F32, tag="pb")
                nc.sync.dma_start(
                    out=pb_t[:], in_=pb_d[layer].rearrange("b p c -> p b c")
                )
                fb_bv = small_p.tile([P, E], F32, tag="fb_bv")
                nc.sync.dma_start(out=fb_bv[:], in_=fb_d[layer, 0])
                fb_g = small_p.tile([P, E], F32, tag="fb_g")
                nc.sync.dma_start(out=fb_g[:], in_=fb_d[layer, 1])
                fb_b = small_p.tile([P, E], F32, tag="fb_b")
                nc.sync.dma_start(out=fb_b[:], in_=fb_d[layer, 2])

                # ---- q/k projections -> qT/kT [f, s] (f on partitions) ----
                qkT = []
                for m in range(2):
                    dst = qk_p.tile([P, NE, S], BF16, tag=f"qk{m}")
                    for fc in range(NE):
                        for sh in range(NSH):
                            ps = ps_s.tile([P, FREE], F32, tag="ps_s")
                            for ec in range(NE):
                                nc.tensor.matmul(
                                    ps[:],
                                    w_t[m][:, ec, fc * P : (fc + 1) * P],
                                    xT_cur[:, ec, sh * FREE : (sh + 1) * FREE],
                                    start=(ec == 0),
                                    stop=(ec == NE - 1),
                                )
                            nc.scalar.activation(
                                dst[:, fc, sh * FREE : (sh + 1) * FREE],
                                ps[:],
                                AF.Identity,
                                bias=pb_t[:, m, fc : fc + 1],
                            )
                    qkT.append(dst)
                qT, kT = qkT

                # ---- v projection -> [s, (h, d|1)] with ones column ----
                v_t = []
                for sc in range(NS):
                    vt = v_p.tile([P, H, D + 1], BF16, tag="v")
                    nc.scalar.copy(vt[:, :, D], ones_c[:])
                    ps = ps_s.tile([P, FREE], F32, tag="ps_s")
                    for ec in range(NE):
                        nc.tensor.matmul(
                            ps[:],
                            xT_cur[:, ec, sc * P : (sc + 1) * P],
                            w_t[2][:, ec, :],
                            start=(ec == 0),
                            stop=(ec == NE - 1),
                        )
                    nc.vector.tensor_tensor(
                        vt[:, :, 0:D],
                        ps[:].rearrange("p (h d) -> p h d", d=D),
                        fb_bv[:].rearrange("p (h d) -> p h d", d=D),
                        OP.add,
                    )
                    v_t.append(vt)

                # ---- attention ----
                at_t = at_p.tile([P, NE, S], MM_DT, tag="at")
                for h in range(H):
                    hc, hr = h // 2, (h % 2) * D
                    for sh in range(NSH):
                        sq = slice(sh * FREE, (sh + 1) * FREE)
                        epairs = []
                        for kcp in range(NS // 2):
                            e = exp_p.tile([P, 2, FREE], BF16, tag="exp", bufs=8)
                            for half in range(2):
                                kc = 2 * kcp + half
                                ps = ps_s.tile([P, FREE], F32, tag="ps_s")
                                nc.tensor.matmul(
                                    ps[:],
                                    kT[hr : hr + D, hc, kc * P : (kc + 1) * P],
                                    qT[hr : hr + D, hc, sq],
                                    start=True,
                                    stop=True,
                                )
                                nc.vector.tensor_tensor(
                                    e[:, half, :], ps[:], adj_t[kc][:, sq], OP.mult
                                )
                            nc.scalar.activation(e[:], e[:], AF.Exp)
                            epairs.append(e)
                        po = ps_o.tile([D + 1, FREE], F32, tag="ps_o")
                        for kc in range(NS):
                            nc.tensor.matmul(
                                po[:],
                                v_t[kc][:, h, :],
                                epairs[kc // 2][:, kc % 2, :],
                                start=(kc == 0),
                                stop=(kc == NS - 1),
                            )
                        drow = scr_p.tile([1, FREE], F32, tag="drow")
                        nc.scalar.copy(drow[:], po[D : D + 1, :])
                        rrow = scr_p.tile([1, FREE], F32, tag="rrow")
                        nc.vector.reciprocal_approx_fast(rrow[:], drow[:])
                        rbc = scr_p.tile([D, FREE], F32, tag="rbc")
                        nc.gpsimd.partition_broadcast(rbc[:], rrow[:])
                        nc.vector.scalar_tensor_tensor(
                            at_t[hr : hr + D, hc, sq],
                            po[0:D, :],
                            1.0,
                            rbc[:],
                            OP.mult,
                            OP.mult,
                        )

                # ---- out projection + residual + LayerNorm ----
                ssum = small_p.tile([P, NS], F32, tag="ssum")
                ssq = small_p.tile([P, NS], F32, tag="ssq")
                for sc in range(NS):
                    ps = ps_s.tile([P, FREE], F32, tag="ps_s")
                    for ec in range(NE):
                        nc.tensor.matmul(
                            ps[:],
                            at_t[:, ec, sc * P : (sc + 1) * P],
                            w_t[3][:, ec, :],
                            start=(ec == 0),
                            stop=(ec == NE - 1),
                        )
                    # residual in place: xn <- proj + xn, accumulating row sums
                    nc.vector.scalar_tensor_tensor(
                        xn_t[:, sc, :],
                        ps[:],
                        1.0,
                        xn_t[:, sc, :],
                        OP.mult,
                        OP.add,
                        accum_out=ssum[:, sc : sc + 1],
                    )
                    sq_scr = scr_p.tile([P, E], F32, tag="sqscr", bufs=1)
                    nc.scalar.activation(
                        sq_scr[:],
                        xn_t[:, sc, :],
                        AF.Square,
                        accum_out=ssq[:, sc : sc + 1],
                    )

                negmu = small_p.tile([P, NS], F32, tag="negmu")
                nc.vector.tensor_scalar_mul(negmu[:], ssum[:], -1.0 / E)
                musq = small_p.tile([P, NS], F32, tag="musq")
                nc.vector.tensor_tensor(musq[:], negmu[:], negmu[:], OP.mult)
                sd = small_p.tile([P, NS], F32, tag="sd")
                nc.vector.scalar_tensor_tensor(
                    sd[:], ssq[:], 1.0 / E, musq[:], OP.mult, OP.subtract
                )
                nc.scalar.activation(sd[:], sd[:], AF.Sqrt, bias=eps_t[:])
                rstd = small_p.tile([P, NS], F32, tag="rstd")
                nc.vector.reciprocal_approx_fast(rstd[:], sd[:])

                for sc in range(NS):
                    xsc = xn_t[:, sc, :]
                    nc.vector.tensor_scalar(
                        xsc,
                        xsc,
                        negmu[:, sc : sc + 1],
                        rstd[:, sc : sc + 1],
                        op0=OP.add,
                        op1=OP.mult,
                    )
                    nc.vector.scalar_tensor_tensor(
                        xsc, xsc, 1.0, fb_g[:], OP.mult, OP.mult
                    )
                    nc.vector.tensor_tensor(xsc, xsc, fb_b[:], OP.add)
                    if layer == L - 1:
                        nc.sync.dma_start(
                            out=out_d[:].rearrange("(c p) e -> p c e", p=P)[:, sc, :],
                            in_=xn_t[:, sc, :],
                        )

                # ---- transpose x_new -> xT for next layer ----
                if layer < L - 1:
                    xT_next = xt_p.tile([P, NE, S], MM_DT, tag="xt")
                    for ec in range(NE):
                        for sc in range(NS):
                            pt = ps_t.tile([P, P], F32, tag="ps_t")
                            nc.tensor.transpose(
                                pt[:], xn_t[:, sc, ec * P : (ec + 1) * P], ident[:]
                            )
                            nc.scalar.copy(
                                xT_next[:, ec, sc * P : (sc + 1) * P], pt[:]
                            )
                    xT_cur = xT_next

    nc.compile()
    return nc


_NC = None
LAST_RESULT = None


def _get_nc():
    global _NC
    if _NC is None:
        _NC = build_nc()
    return _NC


def prep_inputs(x, adj, Wq, bq, Wk, bk, Wv, bv, Wo, bo, gamma, beta):
    """Host-side layout prep. Returns per-core input maps."""
    f32 = np.float32
    x = np.asarray(x, f32)
    adj = np.asarray(adj, f32)
    Wq = np.asarray(Wq, f32)
    bq = np.asarray(bq, f32)
    Wk = np.asarray(Wk, f32)
    bk = np.asarray(bk, f32)
    Wv = np.asarray(Wv, f32)
    bv = np.asarray(bv, f32)
    Wo = np.asarray(Wo, f32)
    bo = np.asarray(bo, f32)
    gamma = np.asarray(gamma, f32)
    beta = np.asarray(beta, f32)

    inv = f32(1.0 / math.sqrt(D))
    # einsum('bse,fe->bsf') => out = x @ W.T, contraction over e. lhsT layout
    # wants W.T = [e, f]. Scale folded into Wq/bq.
    wts = np.stack(
        [
            (Wq * inv).transpose(0, 2, 1),
            Wk.transpose(0, 2, 1),
            Wv.transpose(0, 2, 1),
            Wo.transpose(0, 2, 1),
        ],
        axis=1,
    ).astype(f32)  # [L, 4, e, f]
    wts = np.ascontiguousarray(wts)

    # per-partition bias columns for qT/kT evac: [L, 2, 128, chunk]
    pb = np.stack(
        [
            (bq * inv).reshape(L, NE, P).transpose(0, 2, 1),
            bk.reshape(L, NE, P).transpose(0, 2, 1),
        ],
        axis=1,
    ).astype(f32)
    pb = np.ascontiguousarray(pb)

    # fold next layer's bo into this layer's beta; layer0 bo into initial xn
    beta_eff = beta.copy()
    beta_eff[: L - 1] += bo[1:]
    fb = np.stack(
        [
            np.broadcast_to(bv[:, None, :], (L, P, E)),
            np.broadcast_to(gamma[:, None, :], (L, P, E)),
            np.broadcast_to(beta_eff[:, None, :], (L, P, E)),
        ],
        axis=1,
    ).astype(f32)
    fb = np.ascontiguousarray(fb)

    adjT = np.ascontiguousarray(adj.T)

    in_maps = []
    for b in range(x.shape[0]):
        in_maps.append(
            {
                "xT": np.ascontiguousarray(x[b].T),
                "xn": np.ascontiguousarray(x[b] + bo[0][None, :]),
                "wts": wts,
                "pb": pb,
                "fb": fb,
                "adjT": adjT,
            }
        )
    return in_maps


def kernel(x, adj, Wq, bq, Wk, bk, Wv, bv, Wo, bo, gamma, beta):
    global LAST_RESULT
    nc = _get_nc()
    in_maps = prep_inputs(x, adj, Wq, bq, Wk, bk, Wv, bv, Wo, bo, gamma, beta)
    n = len(in_maps)
    trace = os.environ.get("KERNEL_TRACE", "0") == "1"
    res = run_bass_kernel_spmd(nc, in_maps, list(range(n)), trace=trace)
    LAST_RESULT = res
    out = np.stack([res.results[b]["out"] for b in range(n)]).astype(np.float32)
    return out

